# revision 20
# baseline (speedup 1.0000x reference)
"""Trainium2 Bass kernel for the FFT-smoothing (causal EMA) module.

The reference computes, via FFT cross-correlation (ortho norm, L=4096):
    y[b,t,c]   = sum_{m<=t} s^(t-m) * x[b,m,c]          (causal EMA)
    out[b,t,c] = 0.5*x[b,t,c] + 0.5*s^(t+1)*v0[c] + (0.5*(1-s)/64*v1[c]) * y[b,t,c]
with s = sigmoid(smoothing_weight).

On device we compute y as a blocked triangular matmul over 128-row blocks:
    psum_i = T_K @ xs_i + sum_j (s^(jK) u' v^T) @ xs_{i-j} + E_i
where xs = x * G1[c] (channel prescale folded into the matmul inputs),
T_K[a,b] = s^(a-b) (b<=a), the carry blocks are exact off-diagonal blocks
of the full triangular matrix, and E_i[a,c] = 0.5*s^(iK+a+1)*v0[c] is the
rank-1 v0 term.  Carry blocks / E terms whose largest entry is below 1e-12
are dropped (identical result in fp32; for s=0.607 only the j=1 carry and
the i=0 E term survive).

Sharding: pure data-parallel over batch B=32 -> 4 batches per core x 8 cores.
"""

import numpy as np

N_CORES = 8
B, N, C = 32, 2048, 256
K = 128
NBLK = N // K          # 16
BSH = B // N_CORES     # 4 batches per core
CARRY_EPS = 1e-12

_compiled = {}


def _build(ncarry: int, ne: int):
    """Build + compile the SPMD kernel with `ncarry` carry depths and
    `ne` per-block E rank-1 terms."""
    from concourse import bacc, mybir
    import concourse.tile as tile

    F32 = mybir.dt.float32
    F32R = mybir.dt.float32r
    ALU = mybir.AluOpType

    nc = bacc.Bacc("TRN2", target_bir_lowering=False, debug=False,
                   num_devices=N_CORES)

    # x arrives host-pre-permuted to chunk-linear layout:
    # x[n, h, p, i*C+c] = x_orig[n, h*CHB*K + i*K + p, c] -> linear DMAs
    x_ext = nc.dram_tensor("x", [BSH, NBLK // 4, K, 4 * C], F32,
                           kind="ExternalInput").ap()
    # all constants packed into one tensor (single DMA):
    # cols [0:K) tk | [K : K+K*ncarry) tcar_j | [go : go+C) g1 row (f32 bits)
    # | row 0 of [uo : uo+ne*K) up | row 0 of [zo : zo+C) zb
    go = K + K * ncarry
    uo = go + C
    zo = uo + max(ne, 1) * K
    CW = zo + C
    cst_ext = nc.dram_tensor("cst", [K, CW], F32R, kind="ExternalInput").ap()
    out_ext = nc.dram_tensor("out", [BSH, NBLK // 4, K, 4 * C], F32,
                             kind="ExternalOutput").ap()

    CHB = 4                      # blocks per chunk
    NCH = NBLK // CHB            # chunks per batch
    CHW = CHB * C                # chunk free width (1024)

    with tile.TileContext(nc) as tc:
        with (
            tc.tile_pool(name="consts", bufs=1) as cpool,
            tc.tile_pool(name="xin", bufs=10) as xpool,
            tc.tile_pool(name="xsc", bufs=8) as xspool,
            tc.tile_pool(name="outp", bufs=8) as opool,
            tc.tile_pool(name="psum", bufs=8, space="PSUM") as ppool,
        ):
            # one DMA for every constant; slices of this tile feed the
            # matmuls (f32r) and the prescale (g1, bitcast f32)
            cst = cpool.tile([K, CW], F32R)
            nc.scalar.dma_start(cst[:], cst_ext)
            tk_t = cst[:, 0:K]
            tcar_ts = [cst[:, K + j * K:K + (j + 1) * K] for j in range(ncarry)]
            up_t = cst[0:1, uo:uo + max(ne, 1) * K]
            zb_t = cst[0:1, zo:zo + C]
            # replicate g1 (128,C) -> (128, CHW) once on-device
            g1_wide = cpool.tile([K, CHW], F32)
            nc.vector.tensor_copy(
                g1_wide[:].rearrange("p (i c) -> p i c", c=C),
                cst[:, go:go + C].bitcast(F32)
                   .rearrange("p (u c) -> p u c", u=1)
                   .broadcast_to([K, CHB, C]))


            # prescale engine split: ~7/16 chunks on DVE, 9/16 on GpSimd
            # (GpSimd is ~1.55x slower; DVE also carries the epilogue)
            gp_flags = [(k * 9) % 16 < 9 for k in range(16)]

            chunk_idx = 0
            for n in range(BSH):
                xs_hist = []    # xs tiles of this batch's chunks (for carries)
                for h in range(NCH):
                    xt = xpool.tile([K, CHW], F32)
                    nc.sync.dma_start(xt[:], x_ext[n, h])

                    xs = xspool.tile([K, CHW], F32R)
                    near_end = chunk_idx >= BSH * NCH - 3
                    peng = (nc.gpsimd
                            if (gp_flags[chunk_idx % 16] and not near_end)
                            else nc.vector)
                    peng.tensor_tensor(
                        out=xs[:], in0=xt[:], in1=g1_wide[:],
                        op=ALU.mult)
                    chunk_idx += 1
                    xs_hist.append(xs)

                    ot = opool.tile([K, CHW], F32)
                    # For the global last chunk, finish per-block so the
                    # final STT -> out-DMA tail is 1 block, not 4.
                    groups = [(0, 2), (2, 2)]   # 1 PSUM bank per group
                    for g0, gn in groups:
                        ps = ppool.tile([K, gn * C], F32, tag="ps")
                        for k in range(gn):
                            ic = g0 + k
                            i = h * CHB + ic          # block within batch
                            out_ap = ps[:, k * C:(k + 1) * C]
                            mms = [(tk_t, xs, ic)]
                            for j in range(1, ncarry + 1):
                                if i - j >= 0:
                                    mms.append(
                                        (tcar_ts[j - 1],
                                         xs_hist[(i - j) // CHB],
                                         (i - j) % CHB))
                            has_e = i < ne
                            nmm = len(mms) + (1 if has_e else 0)
                            for mi, (lhs_t, src_t, src_i) in enumerate(mms):
                                nc.tensor.matmul(
                                    out_ap,
                                    lhs_t,
                                    src_t[:, src_i * C:(src_i + 1) * C],
                                    start=(mi == 0),
                                    stop=(mi == nmm - 1),
                                )
                            if has_e:
                                nc.tensor.matmul(
                                    out_ap,
                                    up_t[:, i * K:(i + 1) * K],
                                    zb_t,
                                    start=False,
                                    stop=True,
                                )
                        gsl = slice(g0 * C, (g0 + gn) * C)
                        # epilogue: out = 0.5*x + psum
                        nc.vector.scalar_tensor_tensor(
                            out=ot[:, gsl], in0=xt[:, gsl], scalar=0.5,
                            in1=ps[:], op0=ALU.mult, op1=ALU.add)
                        nc.scalar.dma_start(
                            out_ext[n, h, :, gsl], ot[:, gsl])

    nc.compile()
    return nc


def _host_constants(smoothing_weight, v0, v1):
    w = float(np.asarray(smoothing_weight, dtype=np.float64).reshape(-1)[0])
    s = 1.0 / (1.0 + np.exp(-w))
    a = np.arange(K, dtype=np.float64)
    # T_K[b, a] (lhsT layout): s^(a-b) for a >= b
    d = a[None, :] - a[:, None]          # a - b
    with np.errstate(under="ignore"):
        tk = np.where(d >= 0, s ** np.maximum(d, 0.0), 0.0)

        # carry depth: include carry-j while its largest entry s^((j-1)K+1)
        # is above CARRY_EPS
        tcars = []
        for j in range(1, NBLK):
            if s ** ((j - 1) * K + 1) <= CARRY_EPS:
                break
            tcars.append(s ** (j * K + d))          # s^(jK + a - b), full
        ncarry = len(tcars)

        # E rank-1 terms: block i needs it while s^(iK+1) > CARRY_EPS
        ups = []
        for i in range(NBLK):
            if s ** (i * K + 1) <= CARRY_EPS:
                break
            ups.append(s ** (i * K + a + 1.0))
        ne = len(ups)

        g1row = 0.5 * ((1.0 - s) / 64.0) * np.asarray(
            v1, dtype=np.float64).reshape(C)
        zb = 0.5 * np.asarray(v0, dtype=np.float64).reshape(C)

    # pack into the layout _build expects (see cst_ext comment there)
    go = K + K * ncarry
    uo = go + C
    zo = uo + max(ne, 1) * K
    CW = zo + C
    cst = np.zeros((K, CW), dtype=np.float32)
    cst[:, 0:K] = tk.astype(np.float32)
    for j in range(ncarry):
        cst[:, K + j * K:K + (j + 1) * K] = tcars[j].astype(np.float32)
    cst[:, go:go + C] = np.tile(g1row.astype(np.float32).reshape(1, C), (K, 1))
    if ne > 0:
        cst[0, uo:uo + ne * K] = np.concatenate(ups).astype(np.float32)
        cst[0, zo:zo + C] = zb.astype(np.float32)
    return {"cst": cst}, ncarry, ne


def _run(x_in, smoothing_weight, v0, v1, trace=False):
    from concourse.bass_utils import run_bass_kernel_spmd

    x_in = np.ascontiguousarray(np.asarray(x_in, dtype=np.float32))
    consts, ncarry, ne = _host_constants(smoothing_weight, v0, v1)

    key = (ncarry, ne)
    if key not in _compiled:
        _compiled[key] = _build(ncarry, ne)
    nc = _compiled[key]

    # pre-permute to chunk-linear device layout:
    # (B, N, C) -> (B, NCH, K, CHB*C) with x[n,h,p,i*C+c] = x[n, h*512+i*128+p, c]
    NCH, CHB = NBLK // 4, 4
    xp = x_in.reshape(B, NCH, CHB, K, C).transpose(0, 1, 3, 2, 4)
    xp = np.ascontiguousarray(xp).reshape(B, NCH, K, CHB * C)

    in_maps = []
    for core in range(N_CORES):
        m = {"x": xp[core * BSH:(core + 1) * BSH]}
        m.update(consts)
        in_maps.append(m)

    res = run_bass_kernel_spmd(nc, in_maps, core_ids=list(range(N_CORES)),
                               trace=trace)
    out = np.concatenate([np.asarray(r["out"]) for r in res.results], axis=0)
    out = out.reshape(B, NCH, K, CHB, C).transpose(0, 1, 3, 2, 4)
    out = np.ascontiguousarray(out).reshape(B, N, C)
    return out.astype(np.float32, copy=False), res


def kernel(x_in, smoothing_weight, v0, v1):
    out, _ = _run(x_in, smoothing_weight, v0, v1, trace=False)
    return out


# revision 21
# speedup vs baseline: 1.0323x; 1.0323x over previous
"""Trainium2 Bass kernel for the FFT-smoothing (causal EMA) module.

The reference computes, via FFT cross-correlation (ortho norm, L=4096):
    y[b,t,c]   = sum_{m<=t} s^(t-m) * x[b,m,c]          (causal EMA)
    out[b,t,c] = 0.5*x[b,t,c] + 0.5*s^(t+1)*v0[c] + (0.5*(1-s)/64*v1[c]) * y[b,t,c]
with s = sigmoid(smoothing_weight).

On device we compute y as a blocked triangular matmul over 128-row blocks:
    psum_i = T_K @ xs_i + sum_j (s^(jK) u' v^T) @ xs_{i-j} + E_i
where xs = x * G1[c] (channel prescale folded into the matmul inputs),
T_K[a,b] = s^(a-b) (b<=a), the carry blocks are exact off-diagonal blocks
of the full triangular matrix, and E_i[a,c] = 0.5*s^(iK+a+1)*v0[c] is the
rank-1 v0 term.  Carry blocks / E terms whose largest entry is below 1e-12
are dropped (identical result in fp32; for s=0.607 only the j=1 carry and
the i=0 E term survive).

Sharding: pure data-parallel over batch B=32 -> 4 batches per core x 8 cores.
"""

import numpy as np

N_CORES = 8
B, N, C = 32, 2048, 256
K = 128
NBLK = N // K          # 16
BSH = B // N_CORES     # 4 batches per core
CARRY_EPS = 1e-12

_compiled = {}


def _build(ncarry: int, ne: int):
    """Build + compile the SPMD kernel with `ncarry` carry depths and
    `ne` per-block E rank-1 terms."""
    from concourse import bacc, mybir
    import concourse.tile as tile

    F32 = mybir.dt.float32
    F32R = mybir.dt.float32r
    ALU = mybir.AluOpType

    nc = bacc.Bacc("TRN2", target_bir_lowering=False, debug=False,
                   num_devices=N_CORES)

    # x arrives host-pre-permuted to chunk-linear layout:
    # x[n, h, p, i*C+c] = x_orig[n, h*CHB*K + i*K + p, c] -> linear DMAs
    x_ext = nc.dram_tensor("x", [BSH, NBLK // 4, K, 4 * C], F32,
                           kind="ExternalInput").ap()
    # all constants packed into one tensor (single DMA):
    # cols [0:K) tk | [K : K+K*ncarry) tcar_j | [go : go+C) g1 row (f32 bits)
    # | row 0 of [uo : uo+ne*K) up | row 0 of [zo : zo+C) zb
    go = K + K * ncarry
    uo = go + C
    zo = uo + max(ne, 1) * K
    CW = zo + C
    cst_ext = nc.dram_tensor("cst", [K, CW], F32R, kind="ExternalInput").ap()
    out_ext = nc.dram_tensor("out", [BSH, NBLK // 4, K, 4 * C], F32,
                             kind="ExternalOutput").ap()

    CHB = 4                      # blocks per chunk
    NCH = NBLK // CHB            # chunks per batch
    CHW = CHB * C                # chunk free width (1024)

    with tile.TileContext(nc) as tc:
        with (
            tc.tile_pool(name="consts", bufs=1) as cpool,
            tc.tile_pool(name="xin", bufs=10) as xpool,
            tc.tile_pool(name="xsc", bufs=8) as xspool,
            tc.tile_pool(name="outp", bufs=8) as opool,
            tc.tile_pool(name="psum", bufs=4, space="PSUM") as ppool,
        ):
            # one DMA for every constant; slices of this tile feed the
            # matmuls (f32r) and the prescale (g1, bitcast f32)
            cst = cpool.tile([K, CW], F32R)
            nc.scalar.dma_start(cst[:], cst_ext)
            tk_t = cst[:, 0:K]
            tcar_ts = [cst[:, K + j * K:K + (j + 1) * K] for j in range(ncarry)]
            up_t = cst[0:1, uo:uo + max(ne, 1) * K]
            zb_t = cst[0:1, zo:zo + C]
            # replicate g1 (128,C) -> (128, CHW) once on-device
            g1_wide = cpool.tile([K, CHW], F32)
            nc.vector.tensor_copy(
                g1_wide[:].rearrange("p (i c) -> p i c", c=C),
                cst[:, go:go + C].bitcast(F32)
                   .rearrange("p (u c) -> p u c", u=1)
                   .broadcast_to([K, CHB, C]))


            # prescale engine split: ~7/16 chunks on DVE, 9/16 on GpSimd
            # (GpSimd is ~1.55x slower; DVE also carries the epilogue)
            gp_flags = [(k * 9) % 16 < 9 for k in range(16)]

            chunk_idx = 0
            for n in range(BSH):
                xs_hist = []    # xs tiles of this batch's chunks (for carries)
                for h in range(NCH):
                    xt = xpool.tile([K, CHW], F32)
                    nc.sync.dma_start(xt[:], x_ext[n, h])

                    xs = xspool.tile([K, CHW], F32R)
                    is_last = (n == BSH - 1 and h == NCH - 1)
                    peng = (nc.gpsimd
                            if (gp_flags[chunk_idx % 16] and not is_last)
                            else nc.vector)
                    peng.tensor_tensor(
                        out=xs[:], in0=xt[:], in1=g1_wide[:],
                        op=ALU.mult)
                    chunk_idx += 1
                    xs_hist.append(xs)

                    ot = opool.tile([K, CHW], F32)
                    groups = [(0, CHB)]
                    for g0, gn in groups:
                        ps = ppool.tile([K, gn * C], F32, tag="ps")
                        for k in range(gn):
                            ic = g0 + k
                            i = h * CHB + ic          # block within batch
                            out_ap = ps[:, k * C:(k + 1) * C]
                            mms = [(tk_t, xs, ic)]
                            for j in range(1, ncarry + 1):
                                if i - j >= 0:
                                    mms.append(
                                        (tcar_ts[j - 1],
                                         xs_hist[(i - j) // CHB],
                                         (i - j) % CHB))
                            has_e = i < ne
                            nmm = len(mms) + (1 if has_e else 0)
                            for mi, (lhs_t, src_t, src_i) in enumerate(mms):
                                nc.tensor.matmul(
                                    out_ap,
                                    lhs_t,
                                    src_t[:, src_i * C:(src_i + 1) * C],
                                    start=(mi == 0),
                                    stop=(mi == nmm - 1),
                                )
                            if has_e:
                                nc.tensor.matmul(
                                    out_ap,
                                    up_t[:, i * K:(i + 1) * K],
                                    zb_t,
                                    start=False,
                                    stop=True,
                                )
                        gsl = slice(g0 * C, (g0 + gn) * C)
                        # epilogue: out = 0.5*x + psum
                        nc.vector.scalar_tensor_tensor(
                            out=ot[:, gsl], in0=xt[:, gsl], scalar=0.5,
                            in1=ps[:], op0=ALU.mult, op1=ALU.add)
                        nc.scalar.dma_start(
                            out_ext[n, h, :, gsl], ot[:, gsl])

    nc.compile()
    return nc


def _host_constants(smoothing_weight, v0, v1):
    w = float(np.asarray(smoothing_weight, dtype=np.float64).reshape(-1)[0])
    s = 1.0 / (1.0 + np.exp(-w))
    a = np.arange(K, dtype=np.float64)
    # T_K[b, a] (lhsT layout): s^(a-b) for a >= b
    d = a[None, :] - a[:, None]          # a - b
    with np.errstate(under="ignore"):
        tk = np.where(d >= 0, s ** np.maximum(d, 0.0), 0.0)

        # carry depth: include carry-j while its largest entry s^((j-1)K+1)
        # is above CARRY_EPS
        tcars = []
        for j in range(1, NBLK):
            if s ** ((j - 1) * K + 1) <= CARRY_EPS:
                break
            tcars.append(s ** (j * K + d))          # s^(jK + a - b), full
        ncarry = len(tcars)

        # E rank-1 terms: block i needs it while s^(iK+1) > CARRY_EPS
        ups = []
        for i in range(NBLK):
            if s ** (i * K + 1) <= CARRY_EPS:
                break
            ups.append(s ** (i * K + a + 1.0))
        ne = len(ups)

        g1row = 0.5 * ((1.0 - s) / 64.0) * np.asarray(
            v1, dtype=np.float64).reshape(C)
        zb = 0.5 * np.asarray(v0, dtype=np.float64).reshape(C)

    # pack into the layout _build expects (see cst_ext comment there)
    go = K + K * ncarry
    uo = go + C
    zo = uo + max(ne, 1) * K
    CW = zo + C
    cst = np.zeros((K, CW), dtype=np.float32)
    cst[:, 0:K] = tk.astype(np.float32)
    for j in range(ncarry):
        cst[:, K + j * K:K + (j + 1) * K] = tcars[j].astype(np.float32)
    cst[:, go:go + C] = np.tile(g1row.astype(np.float32).reshape(1, C), (K, 1))
    if ne > 0:
        cst[0, uo:uo + ne * K] = np.concatenate(ups).astype(np.float32)
        cst[0, zo:zo + C] = zb.astype(np.float32)
    return {"cst": cst}, ncarry, ne


def _run(x_in, smoothing_weight, v0, v1, trace=False):
    from concourse.bass_utils import run_bass_kernel_spmd

    x_in = np.ascontiguousarray(np.asarray(x_in, dtype=np.float32))
    consts, ncarry, ne = _host_constants(smoothing_weight, v0, v1)

    key = (ncarry, ne)
    if key not in _compiled:
        _compiled[key] = _build(ncarry, ne)
    nc = _compiled[key]

    # pre-permute to chunk-linear device layout:
    # (B, N, C) -> (B, NCH, K, CHB*C) with x[n,h,p,i*C+c] = x[n, h*512+i*128+p, c]
    NCH, CHB = NBLK // 4, 4
    xp = x_in.reshape(B, NCH, CHB, K, C).transpose(0, 1, 3, 2, 4)
    xp = np.ascontiguousarray(xp).reshape(B, NCH, K, CHB * C)

    in_maps = []
    for core in range(N_CORES):
        m = {"x": xp[core * BSH:(core + 1) * BSH]}
        m.update(consts)
        in_maps.append(m)

    res = run_bass_kernel_spmd(nc, in_maps, core_ids=list(range(N_CORES)),
                               trace=trace)
    out = np.concatenate([np.asarray(r["out"]) for r in res.results], axis=0)
    out = out.reshape(B, NCH, K, CHB, C).transpose(0, 1, 3, 2, 4)
    out = np.ascontiguousarray(out).reshape(B, N, C)
    return out.astype(np.float32, copy=False), res


def kernel(x_in, smoothing_weight, v0, v1):
    out, _ = _run(x_in, smoothing_weight, v0, v1, trace=False)
    return out
